# revision 4
# baseline (speedup 1.0000x reference)
"""LDoRA linear kernel for Trainium2, sharded across 8 NeuronCores.

Math: out = (x @ W^T + (x @ A^T) @ B^T * s) * g0 / max(||W + s*B@A||_row, eps)
With s = alpha/rank = 1.0.  Since V = W + s*B@A must be materialized for the
row norms anyway, we fold the low-rank delta into the base GEMM:
    out = (x @ V^T) * row_scale,  row_scale = g0 / max(||V||_row, eps)

Sharding (tensor-parallel, rowwise over out_features):
  - each of the 8 cores owns 512 output rows (rows of W, B, g0)
  - x and A are replicated
  - core computes V^T (bf16) + row scales locally, then the big GEMM
    out^T[o, t] = sum_i V^T[i, o] * x^T[i, t], scaled per-partition (o) at
    PSUM drain.  Host only does layout (transposes / reshape / concat).
"""

import numpy as np

IN_F = 4096
OUT_F = 4096
RANK = 16
TOKENS = 8192
N_CORES = 8
OUT_LOCAL = OUT_F // N_CORES  # 512
NB_I = IN_F // 128  # 32 i-blocks
NB_O = OUT_LOCAL // 128  # 4 o-blocks
T_CHUNK = 512
NT = TOKENS // T_CHUNK  # 16
EPS = 1e-12

_NC_CACHE = None


def _build_nc():
    from contextlib import ExitStack

    import concourse.bacc as bacc
    import concourse.tile as tile
    from concourse import mybir

    bf16 = mybir.dt.bfloat16
    f32 = mybir.dt.float32

    nc = bacc.Bacc()

    xT = nc.dram_tensor("xT", [IN_F, TOKENS], bf16, kind="ExternalInput")
    wT = nc.dram_tensor("wT", [IN_F, OUT_LOCAL], bf16, kind="ExternalInput")
    a = nc.dram_tensor("a", [RANK, IN_F], bf16, kind="ExternalInput")
    bT = nc.dram_tensor("bT", [RANK, OUT_LOCAL], bf16, kind="ExternalInput")
    g0p = nc.dram_tensor("g0p", [128, NB_O], f32, kind="ExternalInput")
    outT = nc.dram_tensor("outT", [OUT_LOCAL, TOKENS], bf16, kind="ExternalOutput")

    with tile.TileContext(nc) as tc, ExitStack() as ctx:
        const = ctx.enter_context(tc.tile_pool(name="const", bufs=1))
        work = ctx.enter_context(tc.tile_pool(name="work", bufs=3))
        xpool = ctx.enter_context(tc.tile_pool(name="xpool", bufs=3))
        opool = ctx.enter_context(tc.tile_pool(name="opool", bufs=2))
        psum = ctx.enter_context(tc.tile_pool(name="psum", bufs=3, space="PSUM"))
        npsum = ctx.enter_context(tc.tile_pool(name="npsum", bufs=1, space="PSUM"))

        # ---- persistent SBUF tensors ----
        vT_sb = const.tile([128, NB_I, OUT_LOCAL], bf16)  # V^T, 32KB/partition
        wT_sb = const.tile([128, NB_I, OUT_LOCAL], bf16)
        a_sb = const.tile([RANK, IN_F], bf16)
        bT_sb = const.tile([RANK, OUT_LOCAL], bf16)
        g0_sb = const.tile([128, NB_O], f32)
        ones_sb = const.tile([128, 1], bf16)
        rs_sb = const.tile([128, NB_O], f32)  # row_scale, per-partition per o-block

        nc.sync.dma_start(out=wT_sb, in_=wT.rearrange("(ib p) o -> p ib o", p=128))
        nc.sync.dma_start(out=a_sb, in_=a[:, :])
        nc.sync.dma_start(out=bT_sb, in_=bT[:, :])
        nc.sync.dma_start(out=g0_sb, in_=g0p[:, :])
        nc.vector.memset(ones_sb, 1.0)

        # ---- phase A: V^T = W^T + (B@A)^T in bf16, plus row sum-of-squares ----
        nrm_ps = []
        for ob in range(NB_O):
            t = npsum.tile([128, 1], f32, name=f"nrm_{ob}", tag=f"nrm_{ob}")
            nrm_ps.append(t)

        for ib in range(NB_I):
            ba_ps = psum.tile([128, OUT_LOCAL], f32, name="ba_ps", tag="mm")
            # (B@A)^T block: [128 i, 512 o] = A[:, i-block]^T @ B^T
            nc.tensor.matmul(
                ba_ps,
                lhsT=a_sb[:, ib * 128 : (ib + 1) * 128],
                rhs=bT_sb[:, :],
                start=True,
                stop=True,
            )
            # V^T block (bf16): W^T + BA^T
            nc.vector.tensor_add(vT_sb[:, ib, :], wT_sb[:, ib, :], ba_ps)
            # squares for the row norms (bf16 is plenty: relative err of the
            # 4096-term sum ~ 1e-4)
            vsq = work.tile([128, OUT_LOCAL], bf16, name="vsq")
            nc.vector.tensor_mul(vsq, vT_sb[:, ib, :], vT_sb[:, ib, :])
            # per-o-row sum over the i-partition dim via matmul with ones
            for ob in range(NB_O):
                nc.tensor.matmul(
                    nrm_ps[ob],
                    lhsT=vsq[:, ob * 128 : (ob + 1) * 128],
                    rhs=ones_sb[:, :],
                    start=(ib == 0),
                    stop=(ib == NB_I - 1),
                )

        # row_scale = g0 / max(sqrt(nrm), eps)
        for ob in range(NB_O):
            nc.scalar.sqrt(rs_sb[:, ob : ob + 1], nrm_ps[ob])
        nc.vector.tensor_scalar_max(rs_sb, rs_sb, EPS)
        nc.vector.reciprocal(rs_sb, rs_sb)
        nc.vector.tensor_mul(rs_sb, rs_sb, g0_sb)

        # ---- phase B: out^T[o, t] = V^T^T @ x^T, scaled at drain ----
        xTr = xT.rearrange("(ib p) t -> p ib t", p=128)
        outTr = outT.rearrange("(ob p) t -> p ob t", p=128)
        for tci in range(NT):
            x_sb = xpool.tile([128, NB_I, T_CHUNK], bf16, name="x_sb")
            nc.sync.dma_start(
                out=x_sb, in_=xTr[:, :, tci * T_CHUNK : (tci + 1) * T_CHUNK]
            )
            o_sb = opool.tile([128, NB_O, T_CHUNK], bf16, name="o_sb")
            for ob in range(NB_O):
                out_ps = psum.tile([128, T_CHUNK], f32, name="out_ps", tag="mm")
                for ib in range(NB_I):
                    nc.tensor.matmul(
                        out_ps,
                        lhsT=vT_sb[:, ib, ob * 128 : (ob + 1) * 128],
                        rhs=x_sb[:, ib, :],
                        start=(ib == 0),
                        stop=(ib == NB_I - 1),
                    )
                nc.vector.tensor_scalar_mul(o_sb[:, ob, :], out_ps, rs_sb[:, ob : ob + 1])
            nc.sync.dma_start(
                out=outTr[:, :, tci * T_CHUNK : (tci + 1) * T_CHUNK], in_=o_sb
            )

    nc.compile()
    return nc


def get_nc():
    global _NC_CACHE
    if _NC_CACHE is None:
        _NC_CACHE = _build_nc()
    return _NC_CACHE


def kernel(x, weight, A, B, g0, **_unused):
    import ml_dtypes

    from concourse.bass_utils import run_bass_kernel_spmd

    bf16 = ml_dtypes.bfloat16
    x = np.asarray(x, dtype=bf16)
    weight = np.asarray(weight, dtype=bf16)
    A = np.asarray(A, dtype=bf16)
    B = np.asarray(B, dtype=bf16)
    g0 = np.asarray(g0, dtype=np.float32)

    xT = np.ascontiguousarray(x.T)  # [IN_F, TOKENS], replicated
    in_maps = []
    for c in range(N_CORES):
        sl = slice(c * OUT_LOCAL, (c + 1) * OUT_LOCAL)
        in_maps.append(
            {
                "xT": xT,
                "wT": np.ascontiguousarray(weight[sl].T),
                "a": np.ascontiguousarray(A),
                "bT": np.ascontiguousarray(B[sl].T),
                "g0p": np.ascontiguousarray(g0[sl].reshape(NB_O, 128).T),
            }
        )

    res = run_bass_kernel_spmd(get_nc(), in_maps, core_ids=list(range(N_CORES)))
    outT = np.concatenate([res.results[c]["outT"] for c in range(N_CORES)], axis=0)
    return np.ascontiguousarray(outT.T)


# revision 8
# speedup vs baseline: 1.0156x; 1.0156x over previous
"""LDoRA linear kernel for Trainium2, sharded across 8 NeuronCores.

Math: out = (x @ W^T + (x @ A^T) @ B^T * s) * g0 / max(||W + s*B@A||_row, eps)
With s = alpha/rank = 1.0.  Since V = W + s*B@A must be materialized for the
row norms anyway, we fold the low-rank delta into the base GEMM:
    out = (x @ V^T) * row_scale,  row_scale = g0 / max(||V||_row, eps)

Sharding (tensor-parallel, rowwise over out_features):
  - each of the 8 cores owns 512 output rows (rows of W, B, g0)
  - x and A are replicated
  - core computes V^T (bf16) + row scales locally, then the big GEMM
    out^T[o, t] = sum_i V^T[i, o] * x^T[i, t], scaled per-partition (o) at
    PSUM drain.  Host only does layout (transposes / reshape / concat).
"""

import numpy as np

IN_F = 4096
OUT_F = 4096
RANK = 16
TOKENS = 8192
N_CORES = 8
OUT_LOCAL = OUT_F // N_CORES  # 512
NB_I = IN_F // 128  # 32 i-blocks
NB_O = OUT_LOCAL // 128  # 4 o-blocks
T_CHUNK = 512
NT = TOKENS // T_CHUNK  # 16
EPS = 1e-12

_NC_CACHE = None


def _build_nc():
    from contextlib import ExitStack

    import concourse.bacc as bacc
    import concourse.tile as tile
    from concourse import mybir

    bf16 = mybir.dt.bfloat16
    f32 = mybir.dt.float32

    nc = bacc.Bacc()

    xT = nc.dram_tensor("xT", [IN_F, TOKENS], bf16, kind="ExternalInput")
    wT = nc.dram_tensor("wT", [IN_F, OUT_LOCAL], bf16, kind="ExternalInput")
    a = nc.dram_tensor("a", [RANK, IN_F], bf16, kind="ExternalInput")
    bT = nc.dram_tensor("bT", [RANK, OUT_LOCAL], bf16, kind="ExternalInput")
    g0r = nc.dram_tensor("g0r", [1, OUT_LOCAL], f32, kind="ExternalInput")
    outT = nc.dram_tensor("outT", [OUT_LOCAL, TOKENS], bf16, kind="ExternalOutput")

    with tile.TileContext(nc) as tc, ExitStack() as ctx:
        const = ctx.enter_context(tc.tile_pool(name="const", bufs=1))
        work = ctx.enter_context(tc.tile_pool(name="work", bufs=3))
        xpool = ctx.enter_context(tc.tile_pool(name="xpool", bufs=3))
        opool = ctx.enter_context(tc.tile_pool(name="opool", bufs=2))
        dram = ctx.enter_context(tc.tile_pool(name="dram", bufs=1, space="DRAM"))
        psum = ctx.enter_context(tc.tile_pool(name="psum", bufs=4, space="PSUM"))
        npsum = ctx.enter_context(tc.tile_pool(name="npsum", bufs=1, space="PSUM"))

        # ---- persistent SBUF tensors ----
        vT_sb = const.tile([128, NB_I, OUT_LOCAL], bf16)  # V^T, 32KB/partition
        wT_sb = const.tile([128, NB_I, OUT_LOCAL], bf16)
        a_sb = const.tile([RANK, IN_F], bf16)
        bT_sb = const.tile([RANK, OUT_LOCAL], bf16)
        g0_sb = const.tile([1, OUT_LOCAL], f32)
        ones_sb = const.tile([128, 1], bf16)
        rs_row = const.tile([1, OUT_LOCAL], f32)
        rs_sb = const.tile([128, NB_O], f32)  # row_scale, per-partition per o-block

        nc.sync.dma_start(out=a_sb, in_=a[:, :])
        nc.sync.dma_start(out=bT_sb, in_=bT[:, :])
        nc.sync.dma_start(out=g0_sb, in_=g0r[:, :])
        nc.vector.memset(ones_sb, 1.0)

        # wT load split in quarters and interleaved with the first x-chunk's
        # quarters (emitted below) so neither 4MiB transfer serializes the
        # kernel start on HBM bandwidth.
        wTr = wT.rearrange("(ib p) o -> p ib o", p=128)
        QI = NB_I // 4
        for q in range(4):
            nc.sync.dma_start(
                out=wT_sb[:, q * QI : (q + 1) * QI, :],
                in_=wTr[:, q * QI : (q + 1) * QI, :],
            )

        # ---- phase A: V^T = W^T + (B@A)^T in bf16, plus row sum-of-squares ----
        nrm_ps = npsum.tile([1, OUT_LOCAL], f32)

        for ib in range(NB_I):
            ba_ps = psum.tile([128, OUT_LOCAL], f32, name="ba_ps", tag="mm")
            # (B@A)^T block: [128 i, 512 o] = A[:, i-block]^T @ B^T
            nc.tensor.matmul(
                ba_ps,
                lhsT=a_sb[:, ib * 128 : (ib + 1) * 128],
                rhs=bT_sb[:, :],
                start=True,
                stop=True,
            )
            # V^T block (bf16): W^T + BA^T
            nc.vector.tensor_add(vT_sb[:, ib, :], wT_sb[:, ib, :], ba_ps)
            # squares for the row norms (bf16 is plenty: relative err of the
            # 4096-term sum ~ 1e-4)
            vsq = work.tile([128, OUT_LOCAL], bf16, name="vsq")
            nc.vector.tensor_mul(vsq, vT_sb[:, ib, :], vT_sb[:, ib, :])
            # partition-dim (i) reduction via ones-stationary matmul: [1, 512]
            nc.tensor.matmul(
                nrm_ps,
                lhsT=ones_sb[:, :],
                rhs=vsq,
                start=(ib == 0),
                stop=(ib == NB_I - 1),
            )

        # row_scale = g0 / max(sqrt(nrm), eps), on the [1, 512] row
        nc.scalar.sqrt(rs_row, nrm_ps)
        nc.vector.tensor_scalar_max(rs_row, rs_row, EPS)
        nc.vector.reciprocal(rs_row, rs_row)
        nc.vector.tensor_mul(rs_row, rs_row, g0_sb)
        # relayout [1, 512] -> [128, 4] (o onto partitions) via DRAM roundtrip
        rs_dram = dram.tile([1, OUT_LOCAL], f32)
        nc.sync.dma_start(out=rs_dram, in_=rs_row)
        nc.sync.dma_start(
            out=rs_sb, in_=rs_dram.rearrange("one (j p) -> p (one j)", p=128)
        )

        # ---- phase B: out^T[o, t] = V^T^T @ x^T, scaled at drain ----
        xTr = xT.rearrange("(ib p) t -> p ib t", p=128)
        outTr = outT.rearrange("(ob p) t -> p ob t", p=128)
        for tci in range(NT):
            tsl = slice(tci * T_CHUNK, (tci + 1) * T_CHUNK)
            x_sb = xpool.tile([128, NB_I, T_CHUNK], bf16, name="x_sb")
            # 1MiB quarters: the first matmuls only gate on the first quarter
            for q in range(4):
                nc.sync.dma_start(
                    out=x_sb[:, q * QI : (q + 1) * QI, :],
                    in_=xTr[:, q * QI : (q + 1) * QI, tsl],
                )
            o_sb = opool.tile([128, NB_O, T_CHUNK], bf16, name="o_sb")
            for ob in range(NB_O):
                out_ps = psum.tile([128, T_CHUNK], f32, name="out_ps", tag="mm")
                for ib in range(NB_I):
                    nc.tensor.matmul(
                        out_ps,
                        lhsT=vT_sb[:, ib, ob * 128 : (ob + 1) * 128],
                        rhs=x_sb[:, ib, :],
                        start=(ib == 0),
                        stop=(ib == NB_I - 1),
                    )
                nc.vector.tensor_scalar_mul(o_sb[:, ob, :], out_ps, rs_sb[:, ob : ob + 1])
                if ob % 2 == 1:
                    nc.sync.dma_start(
                        out=outTr[:, ob - 1 : ob + 1, tsl],
                        in_=o_sb[:, ob - 1 : ob + 1, :],
                    )

    nc.compile()
    return nc


def get_nc():
    global _NC_CACHE
    if _NC_CACHE is None:
        _NC_CACHE = _build_nc()
    return _NC_CACHE


def make_in_maps(x, weight, A, B, g0):
    import ml_dtypes

    bf16 = ml_dtypes.bfloat16
    x = np.asarray(x, dtype=bf16)
    weight = np.asarray(weight, dtype=bf16)
    A = np.asarray(A, dtype=bf16)
    B = np.asarray(B, dtype=bf16)
    g0 = np.asarray(g0, dtype=np.float32)

    xT = np.ascontiguousarray(x.T)  # [IN_F, TOKENS], replicated
    in_maps = []
    for c in range(N_CORES):
        sl = slice(c * OUT_LOCAL, (c + 1) * OUT_LOCAL)
        in_maps.append(
            {
                "xT": xT,
                "wT": np.ascontiguousarray(weight[sl].T),
                "a": np.ascontiguousarray(A),
                "bT": np.ascontiguousarray(B[sl].T),
                "g0r": np.ascontiguousarray(g0[sl].reshape(1, OUT_LOCAL)),
            }
        )
    return in_maps


def kernel(x, weight, A, B, g0, **_unused):
    from concourse.bass_utils import run_bass_kernel_spmd

    in_maps = make_in_maps(x, weight, A, B, g0)
    res = run_bass_kernel_spmd(get_nc(), in_maps, core_ids=list(range(N_CORES)))
    outT = np.concatenate([res.results[c]["outT"] for c in range(N_CORES)], axis=0)
    return np.ascontiguousarray(outT.T)
